# revision 4
# baseline (speedup 1.0000x reference)
"""Causal single-head attention on 8 Trainium2 NeuronCores.

Reference computation (per batch b of 16):
    q = x @ Wq; k = x @ Wk; v = x @ Wv        # x [2048, 512], W* [512, 64]
    out = softmax_causal(q @ k.T / 8) @ v     # out [2048, 64]

Sharding: data-parallel over batch, 2 batches per core, weights replicated.

Per-core kernel (batch-local b in {0,1}), bf16 matmul inputs with fp32
PSUM accumulation (rel-err ~5e-3, tolerance 2e-2):
  - host supplies xT = x[b].T in bf16 so the D-contraction sits on
    partitions; host also pre-packs [Wq|Wk] and Wv into partition-major
    bf16 layouts so each weight loads with one contiguous DMA
  - qT/kT: psum[0:64]=qT, psum[64:128]=kT via packed lhsT [Wq|Wk]
  - kT blocks 0..7 copied to partitions 0:64 (klo) and qT copied to
    partitions 64:128 (qd) via SBUF->SBUF DMA so score matmuls can be
    ROW-TILED: the score contraction is only HD=64 deep, so two k-blocks
    run CONCURRENTLY in the 128x128 PE array (tile_position (0,0) and
    (64,0), auto-derived from AP base partitions) -> 2x ST throughput
  - scores TRANSPOSED: ST[k, q] = kT.T @ qT -> psum [128, 1024] per
    block pair (first half: blocks 0..njb/2-1 vs klo; second half:
    blocks njb/2.. vs the kT resident in qk[64:128] + qd)
  - vT via Wv-stationary matmuls, then PE-transposed (bf16, 1 cyc/row)
    to v natural, packed as v1[., j, .] = [v_j | 1]: the ones column
    makes the PV matmul emit the softmax denominator for free
  - exp on ACT straight out of psum in [128, 1024] chunks -> bf16 ptil
  - causal: k-blocks above the diagonal skipped; diagonal blocks get a
    triangular mask multiply and suffix-sliced matmuls
  - oT[65, 512] accumulates [v|1].T @ p~ over k-blocks in psum; row 64
    is the denominator l; the UNNORMALIZED [65, T] is DMA'd out straight
    from PSUM and the host does out = o / l (+ final transpose)
  - DMA issue only on the two HWDGE rings (sync + scalar); x streamed
    in bf16 (half the bytes), first 512 tokens of batch 0 prioritized
"""

import sys

sys.path.insert(0, "/opt/trn_rl_repo")

import numpy as np
import ml_dtypes

B, T, D, HD = 16, 2048, 512, 64
NCORES = 8
BPC = B // NCORES          # batches per core
NQ = T // 512              # 512-wide q chunks per batch
NJ = T // 128              # 128-wide k blocks per batch
ND = D // 128              # 128-deep contraction tiles

_cache = {}


def _build_nc():
    import concourse.bacc as bacc
    import concourse.mybir as mybir
    import concourse.tile as tile

    F32 = mybir.dt.float32
    BF16 = mybir.dt.bfloat16
    AF = mybir.ActivationFunctionType

    nc = bacc.Bacc("TRN2", target_bir_lowering=False, debug=False)

    xt_d = nc.dram_tensor("xt", [BPC, D, T], BF16, kind="ExternalInput")
    wqk_d = nc.dram_tensor("wqk", [128, ND * 128], BF16, kind="ExternalInput")
    wv_d = nc.dram_tensor("wv", [128, ND * HD], BF16, kind="ExternalInput")
    ident_d = nc.dram_tensor("ident", [64, 64], BF16, kind="ExternalInput")
    mask_d = nc.dram_tensor("mask", [128, 128], BF16, kind="ExternalInput")
    ot_d = nc.dram_tensor("ot", [BPC, HD + 1, T], F32, kind="ExternalOutput")

    with tile.TileContext(nc) as tc:
        with (
            tc.tile_pool(name="const", bufs=1) as cpool,
            tc.tile_pool(name="xt", bufs=1) as xtpool,
            tc.tile_pool(name="qk", bufs=2) as qkpool,
            tc.tile_pool(name="qd", bufs=2) as qdpool,
            tc.tile_pool(name="klo", bufs=2) as klopool,
            tc.tile_pool(name="vt", bufs=2) as vtpool,
            tc.tile_pool(name="v1", bufs=2) as v1pool,
            tc.tile_pool(name="pt", bufs=3) as ptpool,
            tc.tile_pool(name="ob", bufs=2) as obpool,
            tc.tile_pool(name="st", bufs=2, space="PSUM") as stpool,
            tc.tile_pool(name="otp", bufs=2, space="PSUM") as otpool,
            tc.tile_pool(name="aux", bufs=2, space="PSUM") as auxpool,
        ):
            # warm the exp table set on ACT immediately (no DMA dependency)
            scratch = cpool.tile([1, 8], F32, tag="scratch")
            nc.vector.memset(scratch[:], 0.0)
            scratch2 = cpool.tile([1, 8], F32, tag="scratch2")
            nc.scalar.activation(scratch2[:], scratch[:], AF.Exp)

            # ---- constants / weights (single contiguous DMAs, sync ring) ----
            ident = cpool.tile([64, 64], BF16, tag="ident")
            nc.sync.dma_start(ident[:], ident_d[:])
            mask = cpool.tile([128, 128], BF16, tag="mask")
            nc.sync.dma_start(mask[:], mask_d[:])
            wqk = cpool.tile([128, ND, 128], BF16, tag="wqk")
            nc.sync.dma_start(wqk[:], wqk_d[:].rearrange("p (d c) -> p d c", d=ND))
            wv = cpool.tile([128, ND, HD], BF16, tag="wv")
            nc.sync.dma_start(wv[:], wv_d[:].rearrange("p (d c) -> p d c", d=ND))

            # ---- x streams: batch 0 tokens 0:512 first, then the rest ----
            xts = {}
            for b in range(BPC):
                for d in range(ND):
                    xts[(b, d)] = xtpool.tile(
                        [128, T], BF16, tag=f"xt{b}{d}", name=f"xt{b}{d}"
                    )
            for d in range(ND):
                nc.sync.dma_start(
                    xts[(0, d)][:, 0:512], xt_d[0, 128 * d:128 * (d + 1), 0:512]
                )
            for d in range(ND):
                nc.scalar.dma_start(
                    xts[(0, d)][:, 512:T], xt_d[0, 128 * d:128 * (d + 1), 512:T]
                )
            for d in range(ND):
                nc.scalar.dma_start(
                    xts[(1, d)][:, :], xt_d[1, 128 * d:128 * (d + 1), :]
                )

            qks, qds, klos, vts, v1s = {}, {}, {}, {}, {}
            for b in range(BPC):
                qks[b] = qkpool.tile([128, T], BF16, tag="qk", name=f"qk{b}")
                qds[b] = qdpool.tile([128, T], BF16, tag="qd", name=f"qd{b}")
                klos[b] = klopool.tile([64, 1024], BF16, tag="klo", name=f"klo{b}")
                vts[b] = vtpool.tile([64, T], BF16, tag="vt", name=f"vt{b}")
                v1s[b] = v1pool.tile(
                    [128, NJ, HD + 1], BF16, tag="v1", name=f"v1{b}"
                )
                nc.vector.memset(v1s[b][:, :, HD:HD + 1], 1.0)

            def emit_proj_q(b, Q):
                """Everything attention chunk (b, Q) will need from tokens
                [512Q, 512Q+512): qT/kT (+partition-shifted copies), v
                transposed to natural layout."""
                s = slice(512 * Q, 512 * (Q + 1))
                qk, qd, klo, vt, v1 = qks[b], qds[b], klos[b], vts[b], v1s[b]

                p = auxpool.tile([128, 512], F32, tag="aux", name="pqk")
                for d in range(ND):
                    nc.tensor.matmul(
                        p[:], wqk[:, d, :], xts[(b, d)][:, s],
                        start=(d == 0), stop=(d == ND - 1),
                    )
                nc.vector.tensor_copy(qk[:, s], p[:])
                # qT duplicated onto partitions 64:128 for the row-tiled
                # second score stream; kT of chunks 0/1 onto partitions 0:64
                nc.sync.dma_start(qd[64:128, s], qk[0:64, s])
                if Q < 2:
                    nc.sync.dma_start(klo[0:64, s], qk[64:128, s])

                pv_ = auxpool.tile([64, 512], F32, tag="aux", name="pvt")
                for d in range(ND):
                    nc.tensor.matmul(
                        pv_[:], wv[:, d, :], xts[(b, d)][:, s],
                        start=(d == 0), stop=(d == ND - 1),
                    )
                nc.vector.tensor_copy(vt[:, s], pv_[:])
                for t2 in range(2 * Q, 2 * Q + 2):
                    p2 = auxpool.tile([128, 128], BF16, tag="aux", name="ptr")
                    for tt in range(2):
                        nc.tensor.transpose(
                            p2[:, 64 * tt:64 * (tt + 1)],
                            vt[:, 128 * (2 * t2 + tt):128 * (2 * t2 + tt + 1)],
                            ident[:],
                        )
                    nc.vector.tensor_copy(
                        v1[:, 2 * t2:2 * t2 + 2, 0:HD],
                        p2[:].rearrange("p (a c) -> p a c", a=2),
                    )

            def emit_attn_q(b, Q):
                """One query chunk. Score matmuls are row-tiled pairs: block
                g (klo, array rows 0:64) runs concurrently with block
                njb/2+g (kT in qk[64:128] vs qd, rows 64:128). PV skewed one
                pair behind."""
                qk, qd, klo, v1 = qks[b], qds[b], klos[b], v1s[b]
                pot = otpool.tile([HD + 1, 512], F32, tag="ot", name="pot")
                njb = 4 * (Q + 1)          # causal k-blocks for this chunk
                half = njb // 2
                jlast = njb - 1
                pending = None

                def w0_of(j):
                    return 128 * (j - 4 * Q) if j >= 4 * Q else 0

                def emit_pv(p_tile, js):
                    for idx, j in enumerate(js):
                        w0 = w0_of(j)
                        nc.tensor.matmul(
                            pot[:, w0:512],
                            v1[:, j, :],
                            p_tile[:, 512 * idx + w0:512 * (idx + 1)],
                            start=(j == 0),
                            stop=(j == jlast),
                        )

                for g in range(half):
                    j1, j2 = g, half + g
                    pst = stpool.tile([128, 1024], F32, tag="st", name="pst")
                    w1, w2 = w0_of(j1), w0_of(j2)
                    nc.tensor.matmul(
                        pst[:, w1:512],
                        klo[0:64, 128 * j1:128 * (j1 + 1)],
                        qk[0:64, 512 * Q + w1:512 * (Q + 1)],
                        start=True, stop=True,
                    )
                    nc.tensor.matmul(
                        pst[:, 512 + w2:1024],
                        qk[64:128, 128 * j2:128 * (j2 + 1)],
                        qd[64:128, 512 * Q + w2:512 * (Q + 1)],
                        start=True, stop=True,
                    )
                    ptil = ptpool.tile([128, 1024], BF16, tag="pt", name="ptil")
                    nc.scalar.activation(
                        ptil[:, w1:1024], pst[:, w1:1024], AF.Exp,
                        scale=1.0 / np.sqrt(HD),
                    )
                    for idx, j in ((0, j1), (1, j2)):
                        if j >= 4 * Q:  # diagonal block: triangular mask
                            w0 = w0_of(j)
                            win = slice(512 * idx + w0, 512 * idx + w0 + 128)
                            nc.vector.tensor_mul(ptil[:, win], ptil[:, win], mask[:])
                    if pending is not None:
                        emit_pv(*pending)
                    pending = (ptil, (j1, j2))
                emit_pv(*pending)

                # unnormalized o (rows 0:64) + denominator l (row 64) out;
                # the host divides
                osb = obpool.tile([HD + 1, 512], F32, tag="ob", name="osb")
                nc.vector.tensor_copy(osb[:], pot[:])
                nc.sync.dma_start(ot_d[b, :, 512 * Q:512 * (Q + 1)], osb[:])

            # ---- emission schedule: projections feed attention per-Q ----
            for b in range(BPC):
                for Q in range(NQ):
                    emit_proj_q(b, Q)
                    emit_attn_q(b, Q)

    nc.compile()
    return nc


def _get_nc():
    if "nc" not in _cache:
        _cache["nc"] = _build_nc()
    return _cache["nc"]


def kernel(x, Wq, Wk, Wv, _trace=False, _trace_kwargs=None):
    from concourse.bass_utils import run_bass_kernel_spmd

    x = np.asarray(x, dtype=np.float32)
    Wq = np.asarray(Wq, dtype=np.float32)
    Wk = np.asarray(Wk, dtype=np.float32)
    Wv = np.asarray(Wv, dtype=np.float32)

    nc = _get_nc()

    bf16 = ml_dtypes.bfloat16
    # [Wq|Wk] packed partition-major: wqk[p, d*128+c] = concat(Wq,Wk)[128d+p, c]
    w2 = np.concatenate([Wq, Wk], axis=1).reshape(ND, 128, 128)
    wqk = np.ascontiguousarray(
        w2.transpose(1, 0, 2).reshape(128, ND * 128)
    ).astype(bf16)
    wv = np.ascontiguousarray(
        Wv.reshape(ND, 128, HD).transpose(1, 0, 2).reshape(128, ND * HD)
    ).astype(bf16)
    ident = np.eye(64, dtype=np.float32).astype(bf16)
    mask = np.triu(np.ones((128, 128), dtype=np.float32)).astype(bf16)

    in_maps = []
    for c in range(NCORES):
        xt = np.ascontiguousarray(
            x[BPC * c:BPC * (c + 1)].transpose(0, 2, 1).astype(bf16)
        )
        in_maps.append(
            {"xt": xt, "wqk": wqk, "wv": wv, "ident": ident, "mask": mask}
        )

    kwargs = dict(_trace_kwargs or {})
    res = run_bass_kernel_spmd(
        nc, in_maps, list(range(NCORES)), trace=_trace, **kwargs
    )

    out = np.empty((B, T, HD), dtype=np.float32)
    for c in range(NCORES):
        ot = res.results[c]["ot"]  # [BPC, HD+1, T] unnormalized + denominator
        o = ot[:, 0:HD, :] / ot[:, HD:HD + 1, :]
        out[BPC * c:BPC * (c + 1)] = o.transpose(0, 2, 1)
    if _trace:
        _cache["last_results"] = res
    return out


# revision 5
# speedup vs baseline: 1.1035x; 1.1035x over previous
"""Causal single-head attention on 8 Trainium2 NeuronCores.

Reference computation (per batch b of 16):
    q = x @ Wq; k = x @ Wk; v = x @ Wv        # x [2048, 512], W* [512, 64]
    out = softmax_causal(q @ k.T / 8) @ v     # out [2048, 64]

Sharding: data-parallel over batch, 2 batches per core, weights replicated.

Per-core kernel (batch-local b in {0,1}), bf16 matmul inputs with fp32
PSUM accumulation (rel-err ~5e-3, tolerance 2e-2):
  - host supplies xT = x[b].T in bf16; one SBUF tile [128, ND, T] per
    batch so a single strided DMA covers all four 128-deep D-tiles
  - qT/kT: psum[0:64]=qT, psum[64:128]=kT via packed lhsT [Wq|Wk]
  - kT blocks 0..7 copied to partitions 0:64 (klo) and qT copied to
    partitions 64:128 (qd) via SBUF->SBUF DMA so score matmuls can be
    ROW-TILED: the score contraction is only HD=64 deep, so two k-blocks
    run CONCURRENTLY in the 128x128 PE array (tile_position (0,0) and
    (64,0), auto-derived from AP base partitions) -> 2x ST throughput
  - scores TRANSPOSED: ST[k, q] = kT.T @ qT -> psum [128, 1024] per
    block pair (first half: blocks 0..njb/2-1 vs klo; second half:
    blocks njb/2.. vs the kT resident in qk[64:128] + qd)
  - v projection COL-TILED across the two batches (vT_b0 -> psum rows
    0:64, vT_b1 -> rows 64:128, concurrent), then PE transposes ROW-
    TILED across batches (vt2[0:64] at array rows 0:64, vt2[64:128] at
    rows 64:128, concurrent) into v natural, packed v1[., j, .] =
    [v_j | 1]; the ones column makes the PV matmul emit the softmax
    denominator for free
  - exp on ACT straight out of psum in [128, 1024] chunks -> bf16 ptil
  - causal: k-blocks above the diagonal skipped; diagonal blocks get a
    triangular mask multiply and suffix-sliced matmuls
  - oT[65, 512] accumulates [v|1].T @ p~ over k-blocks in psum; row 64
    is the denominator l; the UNNORMALIZED [65, T] goes out and the
    host does out = o / l (+ final transpose)
  - DMA issue only on the two HWDGE rings (sync + scalar), ordered so
    the first 512 tokens of both batches land first
"""

import sys

sys.path.insert(0, "/opt/trn_rl_repo")

import numpy as np
import ml_dtypes

B, T, D, HD = 16, 2048, 512, 64
NCORES = 8
BPC = B // NCORES          # batches per core
NQ = T // 512              # 512-wide q chunks per batch
NJ = T // 128              # 128-wide k blocks per batch
ND = D // 128              # 128-deep contraction tiles

_cache = {}


def _build_nc():
    import concourse.bacc as bacc
    import concourse.mybir as mybir
    import concourse.tile as tile

    F32 = mybir.dt.float32
    BF16 = mybir.dt.bfloat16
    AF = mybir.ActivationFunctionType

    nc = bacc.Bacc("TRN2", target_bir_lowering=False, debug=False)

    xt_d = nc.dram_tensor("xt", [BPC, D, T], BF16, kind="ExternalInput")
    wqk_d = nc.dram_tensor("wqk", [128, ND * 128], BF16, kind="ExternalInput")
    wv_d = nc.dram_tensor("wv", [128, ND * HD], BF16, kind="ExternalInput")
    ident_d = nc.dram_tensor("ident", [128, 64], BF16, kind="ExternalInput")
    mask_d = nc.dram_tensor("mask", [128, 128], BF16, kind="ExternalInput")
    ot_d = nc.dram_tensor("ot", [BPC, HD + 1, T], F32, kind="ExternalOutput")

    with tile.TileContext(nc) as tc:
        with (
            tc.tile_pool(name="const", bufs=1) as cpool,
            tc.tile_pool(name="xt", bufs=1) as xtpool,
            tc.tile_pool(name="qk", bufs=2) as qkpool,
            tc.tile_pool(name="qd", bufs=2) as qdpool,
            tc.tile_pool(name="klo", bufs=2) as klopool,
            tc.tile_pool(name="vt", bufs=2) as vtpool,
            tc.tile_pool(name="v1", bufs=2) as v1pool,
            tc.tile_pool(name="pt", bufs=3) as ptpool,
            tc.tile_pool(name="ob", bufs=2) as obpool,
            tc.tile_pool(name="st", bufs=2, space="PSUM") as stpool,
            tc.tile_pool(name="otp", bufs=2, space="PSUM") as otpool,
            tc.tile_pool(name="aux", bufs=2, space="PSUM") as auxpool,
        ):
            # warm the exp table set on ACT immediately (no DMA dependency)
            scratch = cpool.tile([1, 8], F32, tag="scratch")
            nc.vector.memset(scratch[:], 0.0)
            scratch2 = cpool.tile([1, 8], F32, tag="scratch2")
            nc.scalar.activation(scratch2[:], scratch[:], AF.Exp)

            # ---- input DMAs, ordered by need; sync + scalar HWDGE rings ----
            xtc = {}
            for b in range(BPC):
                xtc[b] = xtpool.tile([128, ND, T], BF16, tag=f"xt{b}", name=f"xt{b}")
            xsrc = {
                b: xt_d[b].rearrange("(d p) t -> p d t", p=128) for b in range(BPC)
            }
            nc.sync.dma_start(xtc[0][:, :, 0:512], xsrc[0][:, :, 0:512])
            nc.sync.dma_start(xtc[1][:, :, 0:512], xsrc[1][:, :, 0:512])
            wqk = cpool.tile([128, ND, 128], BF16, tag="wqk")
            nc.scalar.dma_start(wqk[:], wqk_d[:].rearrange("p (d c) -> p d c", d=ND))
            wv = cpool.tile([128, ND, HD], BF16, tag="wv")
            nc.scalar.dma_start(wv[:], wv_d[:].rearrange("p (d c) -> p d c", d=ND))
            ident = cpool.tile([128, 64], BF16, tag="ident")
            nc.sync.dma_start(ident[:], ident_d[:])
            mask = cpool.tile([128, 128], BF16, tag="mask")
            nc.sync.dma_start(mask[:], mask_d[:])
            nc.scalar.dma_start(xtc[0][:, :, 512:T], xsrc[0][:, :, 512:T])
            nc.scalar.dma_start(xtc[1][:, :, 512:T], xsrc[1][:, :, 512:T])

            qks, qds, klos, v1s = {}, {}, {}, {}
            for b in range(BPC):
                qks[b] = qkpool.tile([128, T], BF16, tag="qk", name=f"qk{b}")
                qds[b] = qdpool.tile([128, T], BF16, tag="qd", name=f"qd{b}")
                klos[b] = klopool.tile([64, 1024], BF16, tag="klo", name=f"klo{b}")
                v1s[b] = v1pool.tile(
                    [128, NJ, HD + 1], BF16, tag="v1", name=f"v1{b}"
                )
                nc.vector.memset(v1s[b][:, :, HD:HD + 1], 1.0)
            vt2 = vtpool.tile([128, T], BF16, tag="vt", name="vt2")

            def emit_qkproj(b, Q):
                """qT/kT for tokens [512Q, 512Q+512) of batch b, plus the
                partition-shifted copies the row-tiled score matmuls need."""
                s = slice(512 * Q, 512 * (Q + 1))
                qk = qks[b]
                p = auxpool.tile([128, 512], F32, tag="aux", name="pqk")
                for d in range(ND):
                    nc.tensor.matmul(
                        p[:], wqk[:, d, :], xtc[b][:, d, s],
                        start=(d == 0), stop=(d == ND - 1),
                    )
                nc.vector.tensor_copy(qk[:, s], p[:])
                nc.sync.dma_start(qds[b][64:128, s], qk[0:64, s])
                if Q < 2:
                    nc.sync.dma_start(klos[b][0:64, s], qk[64:128, s])

            def emit_vpair(Q):
                """v for tokens [512Q, 512Q+512) of BOTH batches: projection
                col-tiled (b0 -> psum rows 0:64, b1 -> rows 64:128), PE
                transposes row-tiled, both pairs running concurrently."""
                s = slice(512 * Q, 512 * (Q + 1))
                pvv = auxpool.tile([128, 512], F32, tag="aux", name="pvv")
                for d in range(ND):
                    nc.tensor.matmul(
                        pvv[0:64, :], wv[:, d, :], xtc[0][:, d, s],
                        start=(d == 0), stop=(d == ND - 1),
                    )
                    nc.tensor.matmul(
                        pvv[64:128, :], wv[:, d, :], xtc[1][:, d, s],
                        start=(d == 0), stop=(d == ND - 1),
                    )
                nc.vector.tensor_copy(vt2[:, s], pvv[:])
                for t2 in range(2 * Q, 2 * Q + 2):
                    p2a = auxpool.tile([128, 128], BF16, tag="aux", name="p2a")
                    p2b = auxpool.tile([128, 128], BF16, tag="aux", name="p2b")
                    for tt in range(2):
                        ts_ = slice(128 * (2 * t2 + tt), 128 * (2 * t2 + tt + 1))
                        nc.tensor.transpose(
                            p2a[:, 64 * tt:64 * (tt + 1)],
                            vt2[0:64, ts_], ident[0:64, :],
                        )
                        nc.tensor.transpose(
                            p2b[:, 64 * tt:64 * (tt + 1)],
                            vt2[64:128, ts_], ident[64:128, :],
                        )
                    nc.vector.tensor_copy(
                        v1s[0][:, 2 * t2:2 * t2 + 2, 0:HD],
                        p2a[:].rearrange("p (a c) -> p a c", a=2),
                    )
                    nc.vector.tensor_copy(
                        v1s[1][:, 2 * t2:2 * t2 + 2, 0:HD],
                        p2b[:].rearrange("p (a c) -> p a c", a=2),
                    )

            def emit_attn_q(b, Q):
                """One query chunk. Score matmuls are row-tiled pairs: block
                g (klo, array rows 0:64) runs concurrently with block
                njb/2+g (kT in qk[64:128] vs qd, rows 64:128). PV skewed one
                pair behind."""
                qk, qd, klo, v1 = qks[b], qds[b], klos[b], v1s[b]
                pot = otpool.tile([HD + 1, 512], F32, tag="ot", name="pot")
                njb = 4 * (Q + 1)          # causal k-blocks for this chunk
                half = njb // 2
                jlast = njb - 1
                pending = None

                def w0_of(j):
                    return 128 * (j - 4 * Q) if j >= 4 * Q else 0

                def emit_pv(p_tile, js):
                    for idx, j in enumerate(js):
                        w0 = w0_of(j)
                        nc.tensor.matmul(
                            pot[:, w0:512],
                            v1[:, j, :],
                            p_tile[:, 512 * idx + w0:512 * (idx + 1)],
                            start=(j == 0),
                            stop=(j == jlast),
                        )

                for g in range(half):
                    j1, j2 = g, half + g
                    pst = stpool.tile([128, 1024], F32, tag="st", name="pst")
                    w1, w2 = w0_of(j1), w0_of(j2)
                    nc.tensor.matmul(
                        pst[:, w1:512],
                        klo[0:64, 128 * j1:128 * (j1 + 1)],
                        qk[0:64, 512 * Q + w1:512 * (Q + 1)],
                        start=True, stop=True,
                    )
                    nc.tensor.matmul(
                        pst[:, 512 + w2:1024],
                        qk[64:128, 128 * j2:128 * (j2 + 1)],
                        qd[64:128, 512 * Q + w2:512 * (Q + 1)],
                        start=True, stop=True,
                    )
                    ptil = ptpool.tile([128, 1024], BF16, tag="pt", name="ptil")
                    nc.scalar.activation(
                        ptil[:, w1:1024], pst[:, w1:1024], AF.Exp,
                        scale=1.0 / np.sqrt(HD),
                    )
                    for idx, j in ((0, j1), (1, j2)):
                        if j >= 4 * Q:  # diagonal block: triangular mask
                            w0 = w0_of(j)
                            win = slice(512 * idx + w0, 512 * idx + w0 + 128)
                            nc.vector.tensor_mul(ptil[:, win], ptil[:, win], mask[:])
                    if pending is not None:
                        emit_pv(*pending)
                    pending = (ptil, (j1, j2))
                emit_pv(*pending)

                # unnormalized o (rows 0:64) + denominator l (row 64) out;
                # the host divides
                osb = obpool.tile([HD + 1, 512], F32, tag="ob", name="osb")
                nc.vector.tensor_copy(osb[:], pot[:])
                nc.sync.dma_start(ot_d[b, :, 512 * Q:512 * (Q + 1)], osb[:])

            # ---- emission schedule: projections one chunk ahead ----
            emit_qkproj(0, 0)
            emit_qkproj(1, 0)
            emit_vpair(0)
            emit_attn_q(0, 0)
            for Q in range(1, NQ):
                emit_qkproj(0, Q)
                emit_qkproj(1, Q)
                emit_vpair(Q)
                emit_attn_q(0, Q)
            for Q in range(NQ):
                emit_attn_q(1, Q)

    nc.compile()
    return nc


def _get_nc():
    if "nc" not in _cache:
        _cache["nc"] = _build_nc()
    return _cache["nc"]


def kernel(x, Wq, Wk, Wv, _trace=False, _trace_kwargs=None):
    from concourse.bass_utils import run_bass_kernel_spmd

    x = np.asarray(x, dtype=np.float32)
    Wq = np.asarray(Wq, dtype=np.float32)
    Wk = np.asarray(Wk, dtype=np.float32)
    Wv = np.asarray(Wv, dtype=np.float32)

    nc = _get_nc()

    bf16 = ml_dtypes.bfloat16
    # [Wq|Wk] packed partition-major: wqk[p, d*128+c] = concat(Wq,Wk)[128d+p, c]
    w2 = np.concatenate([Wq, Wk], axis=1).reshape(ND, 128, 128)
    wqk = np.ascontiguousarray(
        w2.transpose(1, 0, 2).reshape(128, ND * 128)
    ).astype(bf16)
    wv = np.ascontiguousarray(
        Wv.reshape(ND, 128, HD).transpose(1, 0, 2).reshape(128, ND * HD)
    ).astype(bf16)
    eye = np.eye(64, dtype=np.float32)
    ident = np.concatenate([eye, eye], axis=0).astype(bf16)
    mask = np.triu(np.ones((128, 128), dtype=np.float32)).astype(bf16)

    in_maps = []
    for c in range(NCORES):
        xt = np.ascontiguousarray(
            x[BPC * c:BPC * (c + 1)].transpose(0, 2, 1).astype(bf16)
        )
        in_maps.append(
            {"xt": xt, "wqk": wqk, "wv": wv, "ident": ident, "mask": mask}
        )

    kwargs = dict(_trace_kwargs or {})
    res = run_bass_kernel_spmd(
        nc, in_maps, list(range(NCORES)), trace=_trace, **kwargs
    )

    out = np.empty((B, T, HD), dtype=np.float32)
    for c in range(NCORES):
        ot = res.results[c]["ot"]  # [BPC, HD+1, T] unnormalized + denominator
        o = ot[:, 0:HD, :] / ot[:, HD:HD + 1, :]
        out[BPC * c:BPC * (c + 1)] = o.transpose(0, 2, 1)
    if _trace:
        _cache["last_results"] = res
    return out
